# revision 14
# baseline (speedup 1.0000x reference)
"""Distributed Trainium2 Bass kernel for multi-head causal cross-attention.

Reference computation (B=2, T=2048, E=1024, H=16, d=64):
    q = x @ Wq + bq ; k = y @ Wk + bk ; v = y @ Wv + bv      (per-head reshape)
    att = softmax(q k^T / sqrt(d) + causal_mask)
    out = (att v) @ Wo + bo

Sharding over 8 NeuronCores: data-parallel on batch (2 groups of 4 cores),
tensor-parallel on heads (4 heads = 256 channels per core).  Each core
computes a partial output projection; the host sums the 4 partials per batch
(the unshard step for tensor-parallel partial sums) and adds the output bias.
No on-device collectives are needed.

Per-core dataflow (bf16 operands, fp32 PSUM accumulation):
  - host passes x^T / y^T (bf16) in SLAB-MAJOR layout [128, slab(4) x e(8) x 512]
    so the kernel can start computing on tq/tk slab 0 after ~2 MB of input
    DMA instead of waiting for the full 8 MB; weights wk/wq/wv ride ahead
    of the input slabs on the HWDGE FIFO, wo rides between slab 2 and 3
  - Q^T,K^T = W^T x^T (W stationary), evicted bf16 with fused bias add
  - V in an augmented layout [tk, 4*65]: per head 64 value columns plus a
    ones column, so the PV matmul (M=65) also emits the softmax denominator
    as PSUM row 64
  - scores computed transposed (S^T: tk on partitions, tq free) into a
    2-bank PSUM tile holding both heads of a pair; the two heads' K=64
    matmuls auto-row-tile (tile_position (0,0)/(64,0)) and run concurrently;
    causal blocks skipped; one exp (scale=1/8 fused, no max-subtraction:
    scores ~ N(0,1) after scaling) covers both heads via a segmented AP;
    diagonal 128-blocks are masked on the vector engine with a single
    segmented-AP 0/1 triangular multiply
  - normalization: the PV accumulator banks are evicted to SBUF right after
    the last PV matmul (frees the single-buffered PSUM accumulators for the
    next pair ~2x sooner), then approximate reciprocal of the sums row +
    gpsimd partition-broadcast + fused multiply producing A^T
  - everything is a single software pipeline over tq-blocks J: the non-exp
    tensor work (K/Q/V production for J+1, output projection for J-1) is
    interleaved between attention chunks so the scalar engine (exp) never
    starves; PSUM: 2 x 2-bank score slots + 2 x 1-bank deferred-work slots
    + 2 PV-accumulator banks
  - output DMA via HWDGE (contiguous 256KB per 128-row block); a tiny exp
    at kernel start pulls the ~2.7us activation-table load off the critical
    path

Hardware notes baked in (learned from profiling):
  - bf16 moving operands stream 1 elem/cycle; f32/f32r cost 2 cycles/elem,
    so all matmul operands are bf16 (fp32 PSUM accumulation throughout)
  - with host-side transposes no xbar DMAs remain, so inputs load via HWDGE
    (nc.sync) while small constants load via SWDGE (nc.gpsimd) in parallel
  - reciprocal_approx_fast needs an SBUF source (PSUM source breaks it)
"""

import sys

if "/opt/trn_rl_repo" not in sys.path:
    sys.path.insert(0, "/opt/trn_rl_repo")

import numpy as np
import ml_dtypes

import concourse.bacc as bacc
import concourse.mybir as mybir
import concourse.tile as tile
from concourse.bass_utils import run_bass_kernel_spmd

BF16 = mybir.dt.bfloat16
F32 = mybir.dt.float32
AF = mybir.ActivationFunctionType

B, T, E, H = 2, 2048, 1024, 16
D = E // H                  # 64 head dim
N_CORES = 8
CPC = E // 4                # 256 channels per core (4 heads)

_CACHE = {}
LAST_RESULT = None


def _build():
    nc = bacc.Bacc("TRN2", target_bir_lowering=False, debug=False, num_devices=N_CORES)

    xt = nc.dram_tensor("xt", [128, 16384], BF16, kind="ExternalInput").ap()
    yt = nc.dram_tensor("yt", [128, 16384], BF16, kind="ExternalInput").ap()
    wq = nc.dram_tensor("wq", [128, 8 * CPC], BF16, kind="ExternalInput").ap()
    wk = nc.dram_tensor("wk", [128, 8 * CPC], BF16, kind="ExternalInput").ap()
    wvaug = nc.dram_tensor("wvaug", [128, 8 * 260], BF16, kind="ExternalInput").ap()
    wo = nc.dram_tensor("wo", [128, 2 * E], BF16, kind="ExternalInput").ap()
    bq = nc.dram_tensor("bq", [CPC, 1], F32, kind="ExternalInput").ap()
    bk = nc.dram_tensor("bk", [CPC, 1], F32, kind="ExternalInput").ap()
    bvaug = nc.dram_tensor("bvaug", [1, 260], BF16, kind="ExternalInput").ap()
    btri2 = nc.dram_tensor("btri2", [128, 256], BF16, kind="ExternalInput").ap()
    out = nc.dram_tensor("out", [T, E], BF16, kind="ExternalOutput").ap()

    with tile.TileContext(nc) as tc:
        with (
            nc.allow_low_precision(reason="f32r intermediates; verified <2e-2 end-to-end"),
            tc.tile_pool(name="big", bufs=1) as big,
            tc.tile_pool(name="pt", bufs=6) as ptp,
            tc.tile_pool(name="small", bufs=3) as sm,
            tc.tile_pool(name="zout", bufs=4) as zp,
        ):
            # ---- weights needed first ride the HWDGE FIFO ahead of the
            # input slabs so compute can start as soon as slab 0 lands ----
            wk_b = big.tile([128, 8 * CPC], BF16, tag="wk_b", name="wk_b")
            wq_b = big.tile([128, 8 * CPC], BF16, tag="wq_b", name="wq_b")
            wv_b = big.tile([128, 8 * 260], BF16, tag="wv_b", name="wv_b")
            wo_b = big.tile([128, 2 * E], BF16, tag="wo_b", name="wo_b")
            nc.sync.dma_start(wk_b[:], wk[:, :])
            wk_t = [wk_b[:, CPC * e : CPC * e + CPC] for e in range(8)]
            wq_t = [wq_b[:, CPC * e : CPC * e + CPC] for e in range(8)]
            wv_t = [wv_b[:, 260 * e : 260 * e + 260] for e in range(8)]
            wo_t = [wo_b[:, E * p : E * p + E] for p in range(2)]

            # ---- transposed inputs, slab-major: col = 4096*slab + 512*e + f ----
            yTb = big.tile([128, 16384], BF16, tag="yTb", name="yTb")
            xTb = big.tile([128, 16384], BF16, tag="xTb", name="xTb")

            def load_slab(s):
                nc.sync.dma_start(
                    yTb[:, 4096 * s : 4096 * s + 4096], yt[:, 4096 * s : 4096 * s + 4096]
                )
                nc.sync.dma_start(
                    xTb[:, 4096 * s : 4096 * s + 4096], xt[:, 4096 * s : 4096 * s + 4096]
                )

            nc.sync.dma_start(yTb[:, 0:2048], yt[:, 0:2048])
            nc.sync.dma_start(yTb[:, 2048:4096], yt[:, 2048:4096])
            nc.sync.dma_start(wq_b[:], wq[:, :])
            nc.sync.dma_start(xTb[:, 0:2048], xt[:, 0:2048])
            nc.sync.dma_start(xTb[:, 2048:4096], xt[:, 2048:4096])
            nc.sync.dma_start(wv_b[:], wvaug[:, :])
            load_slab(1)
            load_slab(2)
            nc.sync.dma_start(wo_b[:], wo[:, :])
            load_slab(3)

            def ysl(e, s):
                return yTb[:, 4096 * s + 512 * e : 4096 * s + 512 * e + 512]

            def xsl(e, s):
                return xTb[:, 4096 * s + 512 * e : 4096 * s + 512 * e + 512]

            def ychunk(e, c):
                o = 4096 * (c // 4) + 512 * e + 128 * (c % 4)
                return yTb[:, o : o + 128]

            # ---- constants (SWDGE, parallel with the input stream) ----
            btri_t = big.tile([128, 256], BF16, tag="btri", name="btri")
            nc.gpsimd.dma_start(btri_t[:], btri2[:, :])
            bvaug_t = big.tile([1, 260], BF16, tag="bvaug", name="bvaug")
            nc.gpsimd.dma_start(bvaug_t[:], bvaug[:, :])
            vb_bc = big.tile([128, 260], BF16, tag="vb_bc", name="vb_bc")
            nc.gpsimd.partition_broadcast(vb_bc[:], bvaug_t[0:1, :])

            bq_t = [big.tile([128, 1], F32, tag=f"bq{p}", name=f"bq{p}") for p in range(2)]
            bk_t = [big.tile([128, 1], F32, tag=f"bk{p}", name=f"bk{p}") for p in range(2)]
            for p in range(2):
                nc.gpsimd.dma_start(bq_t[p][:], bq[128 * p : 128 * p + 128, :])
                nc.gpsimd.dma_start(bk_t[p][:], bk[128 * p : 128 * p + 128, :])

            # warm the scalar engine's exp table while DMAs stream
            warm = sm.tile([1, 16], BF16, tag="warm", name="warm")
            nc.scalar.activation(warm[:], btri_t[0:1, 0:16], AF.Exp, scale=0.125)

            KT = [big.tile([128, T], BF16, tag=f"KT{p}", name=f"KT{p}") for p in range(2)]
            QT = [big.tile([128, T], BF16, tag=f"QT{p}", name=f"QT{p}") for p in range(2)]
            AT = [big.tile([128, T], BF16, tag=f"AT{p}", name=f"AT{p}") for p in range(2)]
            V = [big.tile([128, 260], BF16, tag=f"V{c}", name=f"V{c}") for c in range(16)]

            # ---- fused pipeline over tq-blocks J ----
            # Per J: attention chunk loop for both pairs, with the non-exp PE
            # work (K^T/Q^T/V production for J+1, output projection for J-1)
            # interleaved between chunks so the scalar engine (exp) never
            # starves.  J=0's own QKV is a prelude; J=3's outproj is a tail.
            # PSUM banks: s0 2x2 (scores) + d0 2x1 (deferred) + a0/a1 (PV acc).
            with tc.tile_pool(name="psa", bufs=2, space="PSUM") as psa:

                def emit_kq(J, p, which, tag="d0", bufs=2):
                    w_t, dst, bias, src = (
                        (wk_t, KT, bk_t, ysl) if which == "k" else (wq_t, QT, bq_t, xsl)
                    )
                    ps = psa.tile([128, 512], F32, tag=tag, bufs=bufs, name="qk")
                    for e in range(8):
                        nc.tensor.matmul(
                            ps[:],
                            w_t[e][:, 128 * p : 128 * p + 128],
                            src(e, J),
                            start=(e == 0),
                            stop=(e == 7),
                        )
                    nc.vector.tensor_scalar_add(
                        dst[p][:, 512 * J : 512 * J + 512], ps[:], bias[p][:, 0:1]
                    )

                def emit_v(c):
                    psv = psa.tile([128, 260], F32, tag="d0", bufs=2, name="psv")
                    for e in range(8):
                        nc.tensor.matmul(
                            psv[:],
                            ychunk(e, c),
                            wv_t[e][:],
                            start=(e == 0),
                            stop=(e == 7),
                        )
                    # bias + per-head ones columns fused into the eviction
                    nc.vector.tensor_add(V[c][:], psv[:], vb_bc[:])

                def emit_outproj(t):
                    z = zp.tile([128, E], BF16, tag="z", name="z")
                    for eo in range(2):
                        pz = psa.tile([128, 512], F32, tag="d0", bufs=2, name="pz")
                        nc.tensor.matmul(
                            pz[:],
                            AT[0][:, 128 * t : 128 * t + 128],
                            wo_t[0][:, 512 * eo : 512 * eo + 512],
                            start=True,
                            stop=False,
                        )
                        nc.tensor.matmul(
                            pz[:],
                            AT[1][:, 128 * t : 128 * t + 128],
                            wo_t[1][:, 512 * eo : 512 * eo + 512],
                            start=False,
                            stop=True,
                        )
                        if eo == 0:
                            nc.vector.tensor_copy(z[:, 0:512], pz[:])
                        else:
                            nc.scalar.copy(z[:, 512:1024], pz[:])
                    nc.sync.dma_start(out[128 * t : 128 * t + 128, :], z[:])

                # prelude: the minimum for (J=0, p=0) to start -- K both
                # halves (borrowing the free PV-accumulator banks), Q for
                # p=0, V chunk 0.  Everything else joins J=0's deferred list
                # so the first exp fires as soon as the input slabs land.
                emit_kq(0, 0, "k", tag="a0", bufs=1)
                emit_kq(0, 1, "k", tag="a1", bufs=1)
                emit_kq(0, 0, "q")
                emit_v(0)

                tails = []
                for J in range(4):
                    # deferred work: QKV for J+1 and outproj for J-1
                    work = []
                    if J == 0:
                        for c in range(1, 4):
                            work.append(lambda c=c: emit_v(c))
                        work.append(lambda: emit_kq(0, 1, "q"))
                    if J < 3:
                        for p in range(2):
                            work.append(lambda p=p: emit_kq(J + 1, p, "k"))
                            work.append(lambda p=p: emit_kq(J + 1, p, "q"))
                        for c in range(4 * J + 4, 4 * J + 8):
                            work.append(lambda c=c: emit_v(c))
                    if J > 0:
                        for t in range(4 * J - 4, 4 * J):
                            work.append(lambda t=t: emit_outproj(t))

                    nchunks = 4 * J + 4
                    nw = len(work)
                    wi = 0
                    for p in range(2):
                        o0 = psa.tile([65, 512], F32, tag="a0", bufs=1, name="a0")
                        o1 = psa.tile([65, 512], F32, tag="a1", bufs=1, name="a1")
                        # chunks run in PAIRS: both chunks' score matmuls
                        # (64-row-tiled mode) back-to-back, then the deferred
                        # 128-mode work, then all four PV matmuls -- one
                        # tiling-mode switch each way per pair instead of per
                        # chunk (a mode switch drains the PE array)
                        for i0 in range(0, nchunks, 2):
                            exps = []
                            for i in (i0, i0 + 1):
                                r = i - 4 * J
                                full = r < 0
                                lo = 0 if full else 128 * r
                                tqs = slice(512 * J + lo, 512 * J + 512)
                                s0 = psa.tile(
                                    [128, 1024], F32, tag="s0", bufs=2, name="s0"
                                )
                                nc.tensor.matmul(
                                    s0[:, lo:512],
                                    KT[p][0:64, 128 * i : 128 * i + 128],
                                    QT[p][0:64, tqs],
                                    start=True,
                                    stop=True,
                                )
                                nc.tensor.matmul(
                                    s0[:, 512 + lo : 1024],
                                    KT[p][64:128, 128 * i : 128 * i + 128],
                                    QT[p][64:128, tqs],
                                    start=True,
                                    stop=True,
                                )
                                pt0 = ptp.tile(
                                    [128, 1024], BF16, tag="pt0", name="pt0"
                                )
                                if full:
                                    nc.scalar.activation(
                                        pt0[:], s0[:], AF.Exp, scale=0.125
                                    )
                                else:
                                    s3 = s0[:].rearrange("p (s f) -> p s f", s=2)[
                                        :, :, lo:512
                                    ]
                                    p3 = pt0[:].rearrange("p (s f) -> p s f", s=2)[
                                        :, :, lo:512
                                    ]
                                    nc.scalar.activation(p3, s3, AF.Exp, scale=0.125)
                                    # causal band: zero weights where tk > tq
                                    # (single segmented-AP multiply, both heads)
                                    pd = pt0[:].rearrange("p (s f) -> p s f", s=2)[
                                        :, :, lo : lo + 128
                                    ]
                                    bd = btri_t[:].rearrange("p (s f) -> p s f", s=2)
                                    nc.vector.tensor_mul(pd, pd, bd)
                                exps.append((i, lo, pt0))
                            # deferred 128-mode work fills the first exp's window
                            hi_w = nw * (p * nchunks + i0 + 1) // (2 * nchunks)
                            while wi < hi_w:
                                work[wi]()
                                wi += 1
                            for i, lo, pt0 in exps:
                                h0 = 65 * (2 * p)
                                h1 = 65 * (2 * p + 1)
                                nc.tensor.matmul(
                                    o0[0:65, lo:512],
                                    V[i][:, h0 : h0 + 65],
                                    pt0[:, lo:512],
                                    start=(i == 0),
                                    stop=(i == nchunks - 1),
                                )
                                nc.tensor.matmul(
                                    o1[0:65, lo:512],
                                    V[i][:, h1 : h1 + 65],
                                    pt0[:, 512 + lo : 1024],
                                    start=(i == 0),
                                    stop=(i == nchunks - 1),
                                )
                                # a second deferred slot fills the second
                                # exp's window between the PV pairs
                                hi_w = nw * (p * nchunks + i + 1) // (2 * nchunks)
                                while wi < hi_w:
                                    work[wi]()
                                    wi += 1
                        # tail overlap: outproj AT[0]-half matmuls for
                        # t=12..14 only need pair (3,0)'s A^T -- emit them
                        # before the final normalize so the PE stays busy
                        # (and HAM-warm) through it
                        if J == 3 and p == 1:
                            for idx in range(3):
                                if idx < 2:
                                    pz = psa.tile(
                                        [128, 1024], F32, tag="s0", bufs=2, name="pzt"
                                    )
                                    tails.append([pz[:, 0:512], pz[:, 512:1024]])
                                else:
                                    h0 = psa.tile(
                                        [128, 512], F32, tag="d0", bufs=2, name="pz"
                                    )
                                    h1 = psa.tile(
                                        [128, 512], F32, tag="d0", bufs=2, name="pz"
                                    )
                                    tails.append([h0[:], h1[:]])
                            for idx in range(3):
                                t = 12 + idx
                                for eo in range(2):
                                    nc.tensor.matmul(
                                        tails[idx][eo],
                                        AT[0][:, 128 * t : 128 * t + 128],
                                        wo_t[0][:, 512 * eo : 512 * eo + 512],
                                        start=True,
                                        stop=False,
                                    )
                        # normalize: evict the value rows to SBUF (vector and
                        # scalar in parallel -- frees the single-buffered
                        # accumulator banks for the next pair), broadcast the
                        # PSUM denominator row, then take the reciprocal in
                        # place on the broadcast (64 lanes, partition-matched)
                        # gpsimd partition_broadcast (like the custom-DVE
                        # reciprocal) only reads base partition 0, so the
                        # denominator rows get their own partition-0 copies
                        ro0 = sm.tile([1, 512], F32, tag="ro0", name="ro0")
                        ro1 = sm.tile([1, 512], F32, tag="ro1", name="ro1")
                        nc.vector.tensor_copy(ro0[:], o0[64:65, :])
                        nc.scalar.copy(ro1[:], o1[64:65, :])
                        ob0 = sm.tile([64, 512], F32, tag="ob0", name="ob0")
                        ob1 = sm.tile([64, 512], F32, tag="ob1", name="ob1")
                        nc.vector.tensor_copy(ob0[:], o0[0:64, :])
                        nc.scalar.copy(ob1[:], o1[0:64, :])
                        # tail overlap: t=15's outproj surfaces reuse the
                        # accumulator banks just freed by the copies above
                        if J == 3 and p == 1:
                            h0 = psa.tile([128, 512], F32, tag="a0", bufs=1, name="pza")
                            h1 = psa.tile([128, 512], F32, tag="a1", bufs=1, name="pza")
                            tails.append([h0[:], h1[:]])
                            for eo in range(2):
                                nc.tensor.matmul(
                                    tails[3][eo],
                                    AT[0][:, 128 * 15 : 128 * 15 + 128],
                                    wo_t[0][:, 512 * eo : 512 * eo + 512],
                                    start=True,
                                    stop=False,
                                )
                        bs0 = sm.tile([64, 512], F32, tag="bs0", name="bs0")
                        bs1 = sm.tile([64, 512], F32, tag="bs1", name="bs1")
                        nc.gpsimd.partition_broadcast(bs0[:], ro0[0:1, :])
                        nc.gpsimd.partition_broadcast(bs1[:], ro1[0:1, :])
                        # reciprocal in place on the broadcast: 64 lanes in
                        # parallel, partition-matched (the custom-DVE op
                        # breaks on cross-partition APs); first head's
                        # normalize multiply precedes the second reciprocal
                        # so A^T rows 0-63 land as early as possible
                        Js = slice(512 * J, 512 * J + 512)
                        nc.vector.reciprocal_approx_fast(bs0[:], bs0[:])
                        nc.vector.tensor_mul(AT[p][0:64, Js], ob0[:], bs0[:])
                        nc.vector.reciprocal_approx_fast(bs1[:], bs1[:])
                        nc.vector.tensor_mul(AT[p][64:128, Js], ob1[:], bs1[:])

                # tail: AT[1] halves accumulate and evict
                for idx in range(4):
                    t = 12 + idx
                    z = zp.tile([128, E], BF16, tag="z", name="z")
                    for eo in range(2):
                        nc.tensor.matmul(
                            tails[idx][eo],
                            AT[1][:, 128 * t : 128 * t + 128],
                            wo_t[1][:, 512 * eo : 512 * eo + 512],
                            start=False,
                            stop=True,
                        )
                        if eo == 0:
                            nc.vector.tensor_copy(z[:, 0:512], tails[idx][eo])
                        else:
                            nc.scalar.copy(z[:, 512:1024], tails[idx][eo])
                    nc.sync.dma_start(out[128 * t : 128 * t + 128, :], z[:])

    nc.compile()
    return nc


def _get_nc():
    if "nc" not in _CACHE:
        _CACHE["nc"] = _build()
    return _CACHE["nc"]


def _consts():
    if "consts" not in _CACHE:
        bf = ml_dtypes.bfloat16
        btri = (
            np.arange(128)[None, :] >= np.arange(128)[:, None]
        ).astype(np.float32)
        btri2 = np.concatenate([btri, btri], axis=1).astype(bf)
        _CACHE["consts"] = btri2
    return _CACHE["consts"]


def _slabify(a, bf):
    # a: [T, E] float32 -> a.T in slab-major [128, slab(4) x e(8) x 512] bf16
    at = np.ascontiguousarray(a.T)  # [E, T]
    return np.ascontiguousarray(
        at.reshape(8, 128, 4, 512).transpose(1, 2, 0, 3).reshape(128, 16384)
    ).astype(bf)


def kernel(
    x, y, mask, Wq, bq, Wk, bk, Wv, bv, Wo, bo, num_heads, trace=False
):
    global LAST_RESULT
    assert int(num_heads) == H
    x = np.asarray(x, dtype=np.float32)
    y = np.asarray(y, dtype=np.float32)
    Wq = np.asarray(Wq, dtype=np.float32)
    Wk = np.asarray(Wk, dtype=np.float32)
    Wv = np.asarray(Wv, dtype=np.float32)
    Wo = np.asarray(Wo, dtype=np.float32)
    bq = np.asarray(bq, dtype=np.float32)
    bk = np.asarray(bk, dtype=np.float32)
    bv = np.asarray(bv, dtype=np.float32)
    bo = np.asarray(bo, dtype=np.float32)

    bf = ml_dtypes.bfloat16
    btri2 = _consts()

    xtb = [_slabify(x[b], bf) for b in range(B)]
    ytb = [_slabify(y[b], bf) for b in range(B)]

    in_maps = []
    for c in range(N_CORES):
        b = c // 4
        g = c % 4
        cols = slice(CPC * g, CPC * g + CPC)
        wv_s = Wv[:, cols]
        bv_s = bv[cols]
        wvaug = np.zeros((E, 260), dtype=np.float32)
        bvaug = np.zeros((1, 260), dtype=np.float32)
        for h in range(4):
            wvaug[:, 65 * h : 65 * h + 64] = wv_s[:, 64 * h : 64 * h + 64]
            bvaug[0, 65 * h : 65 * h + 64] = bv_s[64 * h : 64 * h + 64]
            bvaug[0, 65 * h + 64] = 1.0
        def arr_w(w):
            # [1024, C] -> [128, 8*C]: partition p holds e-chunks j at cols j*C
            C = w.shape[1]
            return np.ascontiguousarray(
                w.reshape(8, 128, C).transpose(1, 0, 2).reshape(128, 8 * C)
            ).astype(bf)

        wo_s = Wo[cols, :]
        in_maps.append(
            {
                "xt": xtb[b],
                "yt": ytb[b],
                "wq": arr_w(Wq[:, cols]),
                "wk": arr_w(Wk[:, cols]),
                "wvaug": arr_w(wvaug),
                "wo": np.ascontiguousarray(
                    wo_s.reshape(2, 128, E).transpose(1, 0, 2).reshape(128, 2 * E)
                ).astype(bf),
                "bq": np.ascontiguousarray(bq[cols]).reshape(CPC, 1),
                "bk": np.ascontiguousarray(bk[cols]).reshape(CPC, 1),
                "bvaug": bvaug.astype(bf),
                "btri2": btri2,
            }
        )

    nc = _get_nc()
    res = run_bass_kernel_spmd(
        nc, in_maps, core_ids=list(range(N_CORES)), trace=trace
    )
    LAST_RESULT = res

    full = np.zeros((B, T, E), dtype=np.float32)
    for c in range(N_CORES):
        full[c // 4] += res.results[c]["out"].astype(np.float32)
    full += bo
    return full


# revision 15
# speedup vs baseline: 1.0458x; 1.0458x over previous
"""Distributed Trainium2 Bass kernel for multi-head causal cross-attention.

Reference computation (B=2, T=2048, E=1024, H=16, d=64):
    q = x @ Wq + bq ; k = y @ Wk + bk ; v = y @ Wv + bv      (per-head reshape)
    att = softmax(q k^T / sqrt(d) + causal_mask)
    out = (att v) @ Wo + bo

Sharding over 8 NeuronCores: data-parallel on batch (2 groups of 4 cores),
tensor-parallel on heads (4 heads = 256 channels per core).  Each core
computes a partial output projection; the host sums the 4 partials per batch
(the unshard step for tensor-parallel partial sums) and adds the output bias.
No on-device collectives are needed.

Per-core dataflow (bf16 operands, fp32 PSUM accumulation):
  - host passes x^T / y^T (bf16) in SLAB-MAJOR layout [128, slab(4) x e(8) x 512]
    so the kernel can start computing on tq/tk slab 0 after ~2 MB of input
    DMA instead of waiting for the full 8 MB; weights wk/wq/wv ride ahead
    of the input slabs on the HWDGE FIFO, wo rides between slab 2 and 3
  - Q^T,K^T = W^T x^T (W stationary), evicted bf16 with fused bias add
  - V in an augmented layout [tk, 4*65]: per head 64 value columns plus a
    ones column, so the PV matmul (M=65) also emits the softmax denominator
    as PSUM row 64
  - scores computed transposed (S^T: tk on partitions, tq free) into a
    2-bank PSUM tile holding both heads of a pair; the two heads' K=64
    matmuls auto-row-tile (tile_position (0,0)/(64,0)) and run concurrently;
    causal blocks skipped; one exp (scale=1/8 fused, no max-subtraction:
    scores ~ N(0,1) after scaling) covers both heads via a segmented AP;
    diagonal 128-blocks are masked on the vector engine with a single
    segmented-AP 0/1 triangular multiply
  - normalization: the PV accumulator banks are evicted to SBUF right after
    the last PV matmul (frees the single-buffered PSUM accumulators for the
    next pair ~2x sooner), then approximate reciprocal of the sums row +
    gpsimd partition-broadcast + fused multiply producing A^T
  - everything is a single software pipeline over tq-blocks J: the non-exp
    tensor work (K/Q/V production for J+1, output projection for J-1) is
    interleaved between attention chunks so the scalar engine (exp) never
    starves; PSUM: 2 x 2-bank score slots + 2 x 1-bank deferred-work slots
    + 2 PV-accumulator banks
  - output DMA via HWDGE (contiguous 256KB per 128-row block); a tiny exp
    at kernel start pulls the ~2.7us activation-table load off the critical
    path

Hardware notes baked in (learned from profiling):
  - bf16 moving operands stream 1 elem/cycle; f32/f32r cost 2 cycles/elem,
    so all matmul operands are bf16 (fp32 PSUM accumulation throughout)
  - with host-side transposes no xbar DMAs remain, so inputs load via HWDGE
    (nc.sync) while small constants load via SWDGE (nc.gpsimd) in parallel
  - reciprocal_approx_fast needs an SBUF source (PSUM source breaks it)
"""

import sys

if "/opt/trn_rl_repo" not in sys.path:
    sys.path.insert(0, "/opt/trn_rl_repo")

import numpy as np
import ml_dtypes

import concourse.bacc as bacc
import concourse.mybir as mybir
import concourse.tile as tile
from concourse.bass_utils import run_bass_kernel_spmd

BF16 = mybir.dt.bfloat16
F32 = mybir.dt.float32
AF = mybir.ActivationFunctionType

B, T, E, H = 2, 2048, 1024, 16
D = E // H                  # 64 head dim
N_CORES = 8
CPC = E // 4                # 256 channels per core (4 heads)

_CACHE = {}
LAST_RESULT = None


def _build():
    nc = bacc.Bacc("TRN2", target_bir_lowering=False, debug=False, num_devices=N_CORES)

    xt = nc.dram_tensor("xt", [128, 16384], BF16, kind="ExternalInput").ap()
    yt = nc.dram_tensor("yt", [128, 16384], BF16, kind="ExternalInput").ap()
    wq = nc.dram_tensor("wq", [128, 8 * CPC], BF16, kind="ExternalInput").ap()
    wk = nc.dram_tensor("wk", [128, 8 * CPC], BF16, kind="ExternalInput").ap()
    wvaug = nc.dram_tensor("wvaug", [128, 8 * 260], BF16, kind="ExternalInput").ap()
    wo = nc.dram_tensor("wo", [128, 2 * E], BF16, kind="ExternalInput").ap()
    bq = nc.dram_tensor("bq", [CPC, 1], F32, kind="ExternalInput").ap()
    bk = nc.dram_tensor("bk", [CPC, 1], F32, kind="ExternalInput").ap()
    bvaug = nc.dram_tensor("bvaug", [1, 260], BF16, kind="ExternalInput").ap()
    btri2 = nc.dram_tensor("btri2", [128, 256], BF16, kind="ExternalInput").ap()
    out = nc.dram_tensor("out", [T, E], BF16, kind="ExternalOutput").ap()

    with tile.TileContext(nc) as tc:
        with (
            nc.allow_low_precision(reason="f32r intermediates; verified <2e-2 end-to-end"),
            tc.tile_pool(name="big", bufs=1) as big,
            tc.tile_pool(name="pt", bufs=6) as ptp,
            tc.tile_pool(name="small", bufs=3) as sm,
            tc.tile_pool(name="zout", bufs=4) as zp,
        ):
            # ---- weights needed first ride the HWDGE FIFO ahead of the
            # input slabs so compute can start as soon as slab 0 lands ----
            wk_b = big.tile([128, 8 * CPC], BF16, tag="wk_b", name="wk_b")
            wq_b = big.tile([128, 8 * CPC], BF16, tag="wq_b", name="wq_b")
            wv_b = big.tile([128, 8 * 260], BF16, tag="wv_b", name="wv_b")
            wo_b = big.tile([128, 2 * E], BF16, tag="wo_b", name="wo_b")
            nc.sync.dma_start(wk_b[:], wk[:, :])
            wk_t = [wk_b[:, CPC * e : CPC * e + CPC] for e in range(8)]
            wq_t = [wq_b[:, CPC * e : CPC * e + CPC] for e in range(8)]
            wv_t = [wv_b[:, 260 * e : 260 * e + 260] for e in range(8)]
            wo_t = [wo_b[:, E * p : E * p + E] for p in range(2)]

            # ---- transposed inputs, slab-major: col = 4096*slab + 512*e + f ----
            yTb = big.tile([128, 16384], BF16, tag="yTb", name="yTb")
            xTb = big.tile([128, 16384], BF16, tag="xTb", name="xTb")

            def load_slab(s):
                nc.sync.dma_start(
                    yTb[:, 4096 * s : 4096 * s + 4096], yt[:, 4096 * s : 4096 * s + 4096]
                )
                nc.sync.dma_start(
                    xTb[:, 4096 * s : 4096 * s + 4096], xt[:, 4096 * s : 4096 * s + 4096]
                )

            nc.sync.dma_start(yTb[:, 0:2048], yt[:, 0:2048])
            nc.sync.dma_start(yTb[:, 2048:4096], yt[:, 2048:4096])
            nc.sync.dma_start(wq_b[:], wq[:, :])
            nc.sync.dma_start(xTb[:, 0:2048], xt[:, 0:2048])
            nc.sync.dma_start(xTb[:, 2048:4096], xt[:, 2048:4096])
            nc.sync.dma_start(wv_b[:], wvaug[:, :])
            load_slab(1)
            load_slab(2)
            nc.sync.dma_start(wo_b[:], wo[:, :])
            load_slab(3)

            def ysl(e, s):
                return yTb[:, 4096 * s + 512 * e : 4096 * s + 512 * e + 512]

            def xsl(e, s):
                return xTb[:, 4096 * s + 512 * e : 4096 * s + 512 * e + 512]

            def ychunk(e, c):
                o = 4096 * (c // 4) + 512 * e + 128 * (c % 4)
                return yTb[:, o : o + 128]

            # ---- constants (SWDGE, parallel with the input stream) ----
            btri_t = big.tile([128, 256], BF16, tag="btri", name="btri")
            nc.gpsimd.dma_start(btri_t[:], btri2[:, :])
            bvaug_t = big.tile([1, 260], BF16, tag="bvaug", name="bvaug")
            nc.gpsimd.dma_start(bvaug_t[:], bvaug[:, :])
            vb_bc = big.tile([128, 260], BF16, tag="vb_bc", name="vb_bc")
            nc.gpsimd.partition_broadcast(vb_bc[:], bvaug_t[0:1, :])

            bq_t = [big.tile([128, 1], F32, tag=f"bq{p}", name=f"bq{p}") for p in range(2)]
            bk_t = [big.tile([128, 1], F32, tag=f"bk{p}", name=f"bk{p}") for p in range(2)]
            for p in range(2):
                nc.gpsimd.dma_start(bq_t[p][:], bq[128 * p : 128 * p + 128, :])
                nc.gpsimd.dma_start(bk_t[p][:], bk[128 * p : 128 * p + 128, :])

            # warm the scalar engine's exp table while DMAs stream
            warm = sm.tile([1, 16], BF16, tag="warm", name="warm")
            nc.scalar.activation(warm[:], btri_t[0:1, 0:16], AF.Exp, scale=0.125)

            KT = [big.tile([128, T], BF16, tag=f"KT{p}", name=f"KT{p}") for p in range(2)]
            QT = [big.tile([128, T], BF16, tag=f"QT{p}", name=f"QT{p}") for p in range(2)]
            AT = [big.tile([128, T], BF16, tag=f"AT{p}", name=f"AT{p}") for p in range(2)]
            V = [big.tile([128, 260], BF16, tag=f"V{c}", name=f"V{c}") for c in range(16)]

            # ---- fused pipeline over tq-blocks J ----
            # Per J: attention chunk loop for both pairs, with the non-exp PE
            # work (K^T/Q^T/V production for J+1, output projection for J-1)
            # interleaved between chunks so the scalar engine (exp) never
            # starves.  J=0's own QKV is a prelude; J=3's outproj is a tail.
            # PSUM banks: s0 2x2 (scores) + d0 2x1 (deferred) + a0/a1 (PV acc).
            with tc.tile_pool(name="psa", bufs=2, space="PSUM") as psa:

                def emit_kq(J, p, which, tag="d0", bufs=2):
                    w_t, dst, bias, src = (
                        (wk_t, KT, bk_t, ysl) if which == "k" else (wq_t, QT, bq_t, xsl)
                    )
                    ps = psa.tile([128, 512], F32, tag=tag, bufs=bufs, name="qk")
                    for e in range(8):
                        nc.tensor.matmul(
                            ps[:],
                            w_t[e][:, 128 * p : 128 * p + 128],
                            src(e, J),
                            start=(e == 0),
                            stop=(e == 7),
                        )
                    nc.vector.tensor_scalar_add(
                        dst[p][:, 512 * J : 512 * J + 512], ps[:], bias[p][:, 0:1]
                    )

                def emit_v(c):
                    psv = psa.tile([128, 260], F32, tag="d0", bufs=2, name="psv")
                    for e in range(8):
                        nc.tensor.matmul(
                            psv[:],
                            ychunk(e, c),
                            wv_t[e][:],
                            start=(e == 0),
                            stop=(e == 7),
                        )
                    # bias + per-head ones columns fused into the eviction
                    nc.vector.tensor_add(V[c][:], psv[:], vb_bc[:])

                def emit_outproj(t):
                    z = zp.tile([128, E], BF16, tag="z", name="z")
                    for eo in range(2):
                        pz = psa.tile([128, 512], F32, tag="d0", bufs=2, name="pz")
                        nc.tensor.matmul(
                            pz[:],
                            AT[0][:, 128 * t : 128 * t + 128],
                            wo_t[0][:, 512 * eo : 512 * eo + 512],
                            start=True,
                            stop=False,
                        )
                        nc.tensor.matmul(
                            pz[:],
                            AT[1][:, 128 * t : 128 * t + 128],
                            wo_t[1][:, 512 * eo : 512 * eo + 512],
                            start=False,
                            stop=True,
                        )
                        if eo == 0:
                            nc.vector.tensor_copy(z[:, 0:512], pz[:])
                        else:
                            nc.scalar.copy(z[:, 512:1024], pz[:])
                    nc.sync.dma_start(out[128 * t : 128 * t + 128, :], z[:])

                # prelude: the minimum for (J=0, p=0) to start -- K both
                # halves (borrowing the free PV-accumulator banks), Q for
                # p=0, V chunk 0.  Everything else joins J=0's deferred list
                # so the first exp fires as soon as the input slabs land.
                emit_kq(0, 0, "k", tag="a0", bufs=1)
                emit_kq(0, 1, "k", tag="a1", bufs=1)
                emit_kq(0, 0, "q")
                emit_v(0)

                tails = []
                for J in range(4):
                    # deferred work: QKV for J+1 and outproj for J-1
                    work = []
                    # deferred-work schedule rebalanced so J=3 (whose exp
                    # demand exceeds its attention PE work) absorbs the
                    # output projections for t=4..11 and its own K slab
                    if J == 0:
                        for c in range(1, 4):
                            work.append(lambda c=c: emit_v(c))
                        work.append(lambda: emit_kq(0, 1, "q"))
                    if J == 3:
                        for p in range(2):
                            work.append(lambda p=p: emit_kq(3, p, "k"))
                    if J < 3:
                        if J < 2:
                            for p in range(2):
                                work.append(lambda p=p: emit_kq(J + 1, p, "k"))
                        for p in range(2):
                            work.append(lambda p=p: emit_kq(J + 1, p, "q"))
                        for c in range(4 * J + 4, 4 * J + 8):
                            work.append(lambda c=c: emit_v(c))
                    if J == 2:
                        for t in range(0, 4):
                            work.append(lambda t=t: emit_outproj(t))
                    if J == 3:
                        for t in range(4, 12):
                            work.append(lambda t=t: emit_outproj(t))

                    nchunks = 4 * J + 4
                    nw = len(work)
                    wi = 0
                    for p in range(2):
                        o0 = psa.tile([65, 512], F32, tag="a0", bufs=1, name="a0")
                        o1 = psa.tile([65, 512], F32, tag="a1", bufs=1, name="a1")
                        # chunks run in PAIRS: both chunks' score matmuls
                        # (64-row-tiled mode) back-to-back, then the deferred
                        # 128-mode work, then all four PV matmuls -- one
                        # tiling-mode switch each way per pair instead of per
                        # chunk (a mode switch drains the PE array)
                        for i0 in range(0, nchunks, 2):
                            exps = []
                            for i in (i0, i0 + 1):
                                r = i - 4 * J
                                full = r < 0
                                lo = 0 if full else 128 * r
                                tqs = slice(512 * J + lo, 512 * J + 512)
                                s0 = psa.tile(
                                    [128, 1024], F32, tag="s0", bufs=2, name="s0"
                                )
                                nc.tensor.matmul(
                                    s0[:, lo:512],
                                    KT[p][0:64, 128 * i : 128 * i + 128],
                                    QT[p][0:64, tqs],
                                    start=True,
                                    stop=True,
                                )
                                nc.tensor.matmul(
                                    s0[:, 512 + lo : 1024],
                                    KT[p][64:128, 128 * i : 128 * i + 128],
                                    QT[p][64:128, tqs],
                                    start=True,
                                    stop=True,
                                )
                                pt0 = ptp.tile(
                                    [128, 1024], BF16, tag="pt0", name="pt0"
                                )
                                if full:
                                    nc.scalar.activation(
                                        pt0[:], s0[:], AF.Exp, scale=0.125
                                    )
                                else:
                                    s3 = s0[:].rearrange("p (s f) -> p s f", s=2)[
                                        :, :, lo:512
                                    ]
                                    p3 = pt0[:].rearrange("p (s f) -> p s f", s=2)[
                                        :, :, lo:512
                                    ]
                                    nc.scalar.activation(p3, s3, AF.Exp, scale=0.125)
                                    # causal band: zero weights where tk > tq
                                    # (single segmented-AP multiply, both heads)
                                    pd = pt0[:].rearrange("p (s f) -> p s f", s=2)[
                                        :, :, lo : lo + 128
                                    ]
                                    bd = btri_t[:].rearrange("p (s f) -> p s f", s=2)
                                    nc.vector.tensor_mul(pd, pd, bd)
                                exps.append((i, lo, pt0))
                            # deferred 128-mode work fills the first exp's window
                            hi_w = nw * (p * nchunks + i0 + 1) // (2 * nchunks)
                            while wi < hi_w:
                                work[wi]()
                                wi += 1
                            for i, lo, pt0 in exps:
                                h0 = 65 * (2 * p)
                                h1 = 65 * (2 * p + 1)
                                nc.tensor.matmul(
                                    o0[0:65, lo:512],
                                    V[i][:, h0 : h0 + 65],
                                    pt0[:, lo:512],
                                    start=(i == 0),
                                    stop=(i == nchunks - 1),
                                )
                                nc.tensor.matmul(
                                    o1[0:65, lo:512],
                                    V[i][:, h1 : h1 + 65],
                                    pt0[:, 512 + lo : 1024],
                                    start=(i == 0),
                                    stop=(i == nchunks - 1),
                                )
                                # a second deferred slot fills the second
                                # exp's window between the PV pairs
                                hi_w = nw * (p * nchunks + i + 1) // (2 * nchunks)
                                while wi < hi_w:
                                    work[wi]()
                                    wi += 1
                        # tail overlap: outproj AT[0]-half matmuls for
                        # t=12..14 only need pair (3,0)'s A^T -- emit them
                        # before the final normalize so the PE stays busy
                        # (and HAM-warm) through it
                        if J == 3 and p == 1:
                            for idx in range(3):
                                if idx < 2:
                                    pz = psa.tile(
                                        [128, 1024], F32, tag="s0", bufs=2, name="pzt"
                                    )
                                    tails.append([pz[:, 0:512], pz[:, 512:1024]])
                                else:
                                    h0 = psa.tile(
                                        [128, 512], F32, tag="d0", bufs=2, name="pz"
                                    )
                                    h1 = psa.tile(
                                        [128, 512], F32, tag="d0", bufs=2, name="pz"
                                    )
                                    tails.append([h0[:], h1[:]])
                            for idx in range(3):
                                t = 12 + idx
                                for eo in range(2):
                                    nc.tensor.matmul(
                                        tails[idx][eo],
                                        AT[0][:, 128 * t : 128 * t + 128],
                                        wo_t[0][:, 512 * eo : 512 * eo + 512],
                                        start=True,
                                        stop=False,
                                    )
                        # normalize: evict the value rows to SBUF (vector and
                        # scalar in parallel -- frees the single-buffered
                        # accumulator banks for the next pair), broadcast the
                        # PSUM denominator row, then take the reciprocal in
                        # place on the broadcast (64 lanes, partition-matched)
                        # one partition-matched copy per head frees the
                        # accumulator bank; the denominator row then gets a
                        # partition-0 copy (SBUF->SBUF) because gpsimd
                        # partition_broadcast, like the custom-DVE
                        # reciprocal, only reads base partition 0
                        ob0 = sm.tile([65, 512], F32, tag="ob0", name="ob0")
                        ob1 = sm.tile([65, 512], F32, tag="ob1", name="ob1")
                        nc.vector.tensor_copy(ob0[:], o0[0:65, :])
                        nc.scalar.copy(ob1[:], o1[0:65, :])
                        ro0 = sm.tile([1, 512], F32, tag="ro0", name="ro0")
                        ro1 = sm.tile([1, 512], F32, tag="ro1", name="ro1")
                        nc.vector.tensor_copy(ro0[:], ob0[64:65, :])
                        nc.scalar.copy(ro1[:], ob1[64:65, :])
                        # tail overlap: t=15's outproj surfaces reuse the
                        # accumulator banks just freed by the copies above
                        if J == 3 and p == 1:
                            h0 = psa.tile([128, 512], F32, tag="a0", bufs=1, name="pza")
                            h1 = psa.tile([128, 512], F32, tag="a1", bufs=1, name="pza")
                            tails.append([h0[:], h1[:]])
                            for eo in range(2):
                                nc.tensor.matmul(
                                    tails[3][eo],
                                    AT[0][:, 128 * 15 : 128 * 15 + 128],
                                    wo_t[0][:, 512 * eo : 512 * eo + 512],
                                    start=True,
                                    stop=False,
                                )
                        bs0 = sm.tile([64, 512], F32, tag="bs0", name="bs0")
                        bs1 = sm.tile([64, 512], F32, tag="bs1", name="bs1")
                        nc.gpsimd.partition_broadcast(bs0[:], ro0[0:1, :])
                        nc.gpsimd.partition_broadcast(bs1[:], ro1[0:1, :])
                        # reciprocal in place on the broadcast: 64 lanes in
                        # parallel, partition-matched (the custom-DVE op
                        # breaks on cross-partition APs); first head's
                        # normalize multiply precedes the second reciprocal
                        # so A^T rows 0-63 land as early as possible
                        Js = slice(512 * J, 512 * J + 512)
                        nc.vector.reciprocal_approx_fast(bs0[:], bs0[:])
                        nc.vector.tensor_mul(AT[p][0:64, Js], ob0[0:64, :], bs0[:])
                        nc.vector.reciprocal_approx_fast(bs1[:], bs1[:])
                        nc.vector.tensor_mul(AT[p][64:128, Js], ob1[0:64, :], bs1[:])

                # tail: AT[1] accumulates in K=64 halves -- the head-2 rows
                # (0:64) only need the final pair's first normalize multiply,
                # so those matmuls start ~0.7us earlier and keep the PE
                # HAM-warm through the final normalize
                for idx in range(4):
                    t = 12 + idx
                    for eo in range(2):
                        nc.tensor.matmul(
                            tails[idx][eo],
                            AT[1][0:64, 128 * t : 128 * t + 128],
                            wo_t[1][0:64, 512 * eo : 512 * eo + 512],
                            start=False,
                            stop=False,
                        )
                for idx in range(4):
                    t = 12 + idx
                    z = zp.tile([128, E], BF16, tag="z", name="z")
                    for eo in range(2):
                        nc.tensor.matmul(
                            tails[idx][eo],
                            AT[1][64:128, 128 * t : 128 * t + 128],
                            wo_t[1][64:128, 512 * eo : 512 * eo + 512],
                            start=False,
                            stop=True,
                        )
                        if eo == 0:
                            nc.vector.tensor_copy(z[:, 0:512], tails[idx][eo])
                        else:
                            nc.scalar.copy(z[:, 512:1024], tails[idx][eo])
                    nc.sync.dma_start(out[128 * t : 128 * t + 128, :], z[:])

    nc.compile()
    return nc


def _get_nc():
    if "nc" not in _CACHE:
        _CACHE["nc"] = _build()
    return _CACHE["nc"]


def _consts():
    if "consts" not in _CACHE:
        bf = ml_dtypes.bfloat16
        btri = (
            np.arange(128)[None, :] >= np.arange(128)[:, None]
        ).astype(np.float32)
        btri2 = np.concatenate([btri, btri], axis=1).astype(bf)
        _CACHE["consts"] = btri2
    return _CACHE["consts"]


def _slabify(a, bf):
    # a: [T, E] float32 -> a.T in slab-major [128, slab(4) x e(8) x 512] bf16
    at = np.ascontiguousarray(a.T)  # [E, T]
    return np.ascontiguousarray(
        at.reshape(8, 128, 4, 512).transpose(1, 2, 0, 3).reshape(128, 16384)
    ).astype(bf)


def kernel(
    x, y, mask, Wq, bq, Wk, bk, Wv, bv, Wo, bo, num_heads, trace=False
):
    global LAST_RESULT
    assert int(num_heads) == H
    x = np.asarray(x, dtype=np.float32)
    y = np.asarray(y, dtype=np.float32)
    Wq = np.asarray(Wq, dtype=np.float32)
    Wk = np.asarray(Wk, dtype=np.float32)
    Wv = np.asarray(Wv, dtype=np.float32)
    Wo = np.asarray(Wo, dtype=np.float32)
    bq = np.asarray(bq, dtype=np.float32)
    bk = np.asarray(bk, dtype=np.float32)
    bv = np.asarray(bv, dtype=np.float32)
    bo = np.asarray(bo, dtype=np.float32)

    bf = ml_dtypes.bfloat16
    btri2 = _consts()

    xtb = [_slabify(x[b], bf) for b in range(B)]
    ytb = [_slabify(y[b], bf) for b in range(B)]

    in_maps = []
    for c in range(N_CORES):
        b = c // 4
        g = c % 4
        cols = slice(CPC * g, CPC * g + CPC)
        wv_s = Wv[:, cols]
        bv_s = bv[cols]
        wvaug = np.zeros((E, 260), dtype=np.float32)
        bvaug = np.zeros((1, 260), dtype=np.float32)
        for h in range(4):
            wvaug[:, 65 * h : 65 * h + 64] = wv_s[:, 64 * h : 64 * h + 64]
            bvaug[0, 65 * h : 65 * h + 64] = bv_s[64 * h : 64 * h + 64]
            bvaug[0, 65 * h + 64] = 1.0
        def arr_w(w):
            # [1024, C] -> [128, 8*C]: partition p holds e-chunks j at cols j*C
            C = w.shape[1]
            return np.ascontiguousarray(
                w.reshape(8, 128, C).transpose(1, 0, 2).reshape(128, 8 * C)
            ).astype(bf)

        wo_s = Wo[cols, :]
        in_maps.append(
            {
                "xt": xtb[b],
                "yt": ytb[b],
                "wq": arr_w(Wq[:, cols]),
                "wk": arr_w(Wk[:, cols]),
                "wvaug": arr_w(wvaug),
                "wo": np.ascontiguousarray(
                    wo_s.reshape(2, 128, E).transpose(1, 0, 2).reshape(128, 2 * E)
                ).astype(bf),
                "bq": np.ascontiguousarray(bq[cols]).reshape(CPC, 1),
                "bk": np.ascontiguousarray(bk[cols]).reshape(CPC, 1),
                "bvaug": bvaug.astype(bf),
                "btri2": btri2,
            }
        )

    nc = _get_nc()
    res = run_bass_kernel_spmd(
        nc, in_maps, core_ids=list(range(N_CORES)), trace=trace
    )
    LAST_RESULT = res

    full = np.zeros((B, T, E), dtype=np.float32)
    for c in range(N_CORES):
        full[c // 4] += res.results[c]["out"].astype(np.float32)
    full += bo
    return full
